# revision 1
# baseline (speedup 1.0000x reference)
"""Trainium2 Bass kernel for nn_AgentAndNode_embedding (GIN message passing +
per-agent attention pooling), data-parallel over 8 NeuronCores.

Strategy
--------
* Shard by graph: 16 graphs x 1000 nodes per core (edges never cross graphs).
* segment_sum -> dense per-graph adjacency matmul on the PE:
    agg^T[c, i] = sum_s h[s, c] * AT[s, i],  AT[s, i] = #edges (src=s -> dst=i)
  AT is built host-side (bincount over the static edge list) and stored in
  fp8-e4m3 (integers 0..16 are exact); h is fp16 stationary, AT the fp8
  moving operand, f32 PSUM accumulation -> numerically exact aggregation up
  to the fp16 rounding of h.
* GIN MLP in transposed layout (channels on partitions, weights stationary),
  b2 dropped (cancels inside BatchNorm).  BatchNorm statistics via
  bn_stats/bn_aggr per tile, one [64,2] AllReduce per layer for the global
  (cross-core) batch stats; affine+ReLU applied as a single fused ACT pass.
* Attention without materializing k/v:
    u_am = f_m . (Wk_a q_a) / sqrt(K)    (bk.q and max-subtraction cancel in
    softmax), w~ = exp(u/8), Z = sum_m w~, s~ = w~^T f (PE), and
    emb_a = (s~_a / Z_a) Wv_a + bv_a.
"""

import os
import numpy as np
import ml_dtypes

import concourse.bass as bass
import concourse.bacc as bacc
import concourse.tile as tile
from concourse import mybir
from concourse.bass_utils import run_bass_kernel_spmd
from concourse.masks import make_identity

FP16 = mybir.dt.float16
FP8 = mybir.dt.float8e4
F32 = mybir.dt.float32
NP_FP8 = mybir.dt.np(FP8)

NCORES = 8
G = 16          # graphs per core
NN = 1000       # nodes per graph
B = 128         # total graphs
CH = 64         # hidden
CIN = 2         # input channels
CSZ = 125       # nodes per src-chunk
NK = 8          # chunks per graph (8 * 125 = 1000)
NA = 10         # agents
AT_RES = 8      # graphs whose A^T stays SBUF-resident across layers
BN_EPS = 1e-5

AF = mybir.ActivationFunctionType
ALU = mybir.AluOpType

_PROG_CACHE = {}
LAST_RESULTS = None


def _build_program(at_dtype):
    nc = bacc.Bacc("TRN2", target_bir_lowering=False, debug=False,
                   num_devices=NCORES)

    at_d = nc.dram_tensor("at", [G, CSZ, NK, NN], at_dtype, kind="ExternalInput").ap()
    xnat_d = nc.dram_tensor("xnat", [CSZ, G, NK, CIN], FP16, kind="ExternalInput").ap()
    xT_d = nc.dram_tensor("xT", [CIN, G * NN], FP16, kind="ExternalInput").ap()
    w10_d = nc.dram_tensor("w10", [CIN, CH], FP16, kind="ExternalInput").ap()
    wpack_d = nc.dram_tensor("wpack", [CH, 8640], FP16, kind="ExternalInput").ap()
    cpack_d = nc.dram_tensor("cpack", [CH, 29], F32, kind="ExternalInput").ap()
    h3_d = nc.dram_tensor("h3", [G, NN, CH], F32, kind="ExternalOutput").ap()
    embT_d = nc.dram_tensor("embT", [CH, NA, G], F32, kind="ExternalOutput").ap()

    # wpack column offsets
    W1_OFF = {1: 0, 2: 64}
    W2_OFF = {0: 128, 1: 192, 2: 256}
    WKT_OFF = 320      # + 64*a
    WV_OFF = 960       # + 64*a
    WQP_OFF = 1600     # + 64*(a*11 + p)
    # cpack column offsets: b1_l -> l, gamma_l -> 3+l, beta_l -> 6+l,
    # bqT -> 9..18, bvT -> 19..28

    with tile.TileContext(nc) as tc:
        with tc.tile_pool(name="pers", bufs=1) as pers, \
             tc.tile_pool(name="stream", bufs=2) as stream, \
             tc.tile_pool(name="ypool", bufs=3) as ypool, \
             tc.tile_pool(name="small", bufs=2) as small, \
             tc.tile_pool(name="aggps", bufs=2, space="PSUM") as aggps, \
             tc.tile_pool(name="transps", bufs=2, space="PSUM") as transps, \
             tc.tile_pool(name="mlpps", bufs=2, space="PSUM") as mlpps, \
             tc.tile_pool(name="dram", bufs=2, space="DRAM") as dram:

            # ---------- constants / weights ----------
            ident = pers.tile([CH, CH], FP16)
            make_identity(nc, ident[:])
            ones = pers.tile([CSZ, 1], FP16)
            nc.vector.memset(ones[:], 1.0)
            eps_t = pers.tile([CH, 1], F32)
            nc.vector.memset(eps_t[:], BN_EPS)

            wpack = pers.tile([CH, 8640], FP16)
            nc.sync.dma_start(out=wpack[:], in_=wpack_d[:])
            cpack = pers.tile([CH, 29], F32)
            nc.sync.dma_start(out=cpack[:], in_=cpack_d[:])
            w10 = pers.tile([CIN, CH], FP16)
            nc.sync.dma_start(out=w10[:], in_=w10_d[:])
            xnat = pers.tile([CSZ, G, NK, CIN], FP16)
            nc.sync.dma_start(out=xnat[:], in_=xnat_d[:])

            at_res = pers.tile([CSZ, AT_RES, NK, NN], at_dtype)
            for g in range(AT_RES):
                nc.sync.dma_start(out=at_res[:, g, :, :], in_=at_d[g])

            # ---------- persistent state ----------
            hT = pers.tile([CH, G, NN], FP16)
            mT = pers.tile([CH, G, NN], FP16)
            hnat = pers.tile([CSZ, G, NK, CH], FP16)

            def w1(l):
                if l == 0:
                    return w10[:]
                return wpack[:, W1_OFF[l]:W1_OFF[l] + CH]

            def w2(l):
                return wpack[:, W2_OFF[l]:W2_OFF[l] + CH]

            def at_view(g):
                if g < AT_RES:
                    return at_res[:, g, :, :], None
                t = stream.tile([CSZ, NK, NN], at_dtype, tag="at")
                nc.sync.dma_start(out=t[:], in_=at_d[g])
                return t[:], t

            # ================= GIN layers =================
            for l in range(3):
                cin = CIN if l == 0 else CH
                hn = xnat if l == 0 else hnat

                if l > 0:
                    # transpose pass: hT -> hnat (per graph, 8 chunks of 125)
                    for g in range(G):
                        pt = transps.tile([CSZ, NK, CH], FP16, tag="pt")
                        for k in range(NK):
                            nc.tensor.transpose(
                                pt[:, k, :],
                                hT[:, g, k * CSZ:(k + 1) * CSZ],
                                ident[:],
                            )
                        nc.vector.tensor_copy(out=hnat[:, g, :, :], in_=pt[:])

                def emit_agg(g, l=l, cin=cin, hn=hn):
                    at_ap, _ = at_view(g) if l == 0 or g >= AT_RES else (at_res[:, g, :, :], None)
                    aps = aggps.tile([cin, 2, 512], F32, tag="agg")
                    for k in range(NK):
                        for hf in range(2):
                            nc.tensor.matmul(
                                aps[:, hf, 0:500],
                                hn[:, g, k, :],
                                at_ap[:, k, hf * 500:(hf + 1) * 500],
                                start=(k == 0), stop=(k == NK - 1),
                            )
                    if l == 0:
                        xg = stream.tile([CIN, NN], FP16, tag="xg")
                        nc.sync.dma_start(out=xg[:], in_=xT_d[:, g * NN:(g + 1) * NN])
                        hsrc = xg[:]
                    else:
                        hsrc = hT[:, g, :]
                    return aps, hsrc

                def emit_rest(g, state, l=l, cin=cin):
                    aps, hsrc = state
                    z = stream.tile([cin, NN], FP16, tag="z")
                    nc.vector.tensor_tensor(out=z[:], in0=aps[:, :, 0:500], in1=hsrc,
                                            op=ALU.add)
                    for t in range(2):
                        p1 = mlpps.tile([CH, 500], F32, tag="mlp")
                        nc.tensor.matmul(p1[:], w1(l), z[:, t * 500:(t + 1) * 500],
                                         start=True, stop=True)
                        y = ypool.tile([CH, 500], FP16, tag="y")
                        nc.scalar.activation(out=y[:], in_=p1[:], func=AF.Relu,
                                             bias=cpack[:, l:l + 1], scale=1.0)
                        p2 = mlpps.tile([CH, 500], F32, tag="mlp")
                        nc.tensor.matmul(p2[:], w2(l), y[:], start=True, stop=True)
                        nc.vector.bn_stats(out=stats[:, g * 2 + t, :], in_=p2[:])
                        nc.scalar.activation(out=mT[:, g, t * 500:(t + 1) * 500],
                                             in_=p2[:], func=AF.Copy)

                stats = stream.tile([CH, 2 * G, 6], F32, tag="stats")
                state = emit_agg(0)
                for g in range(G):
                    nstate = emit_agg(g + 1) if g + 1 < G else None
                    emit_rest(g, state)
                    state = nstate

                # ---- global BN stats (cross-core) ----
                mv = small.tile([CH, 2], F32, tag="mv")
                nc.vector.bn_aggr(out=mv[:], in_=stats[:])
                red_in = small.tile([CH, 2], F32, tag="red_in")
                nc.vector.tensor_copy(out=red_in[:, 0:1], in_=mv[:, 0:1])
                msq = small.tile([CH, 1], F32, tag="msq")
                nc.vector.tensor_mul(out=msq[:], in0=mv[:, 0:1], in1=mv[:, 0:1])
                nc.vector.tensor_add(out=red_in[:, 1:2], in0=mv[:, 1:2], in1=msq[:])
                din = dram.tile([CH, 2], F32, tag="din")
                dout = dram.tile([CH, 2], F32, tag="dout")
                nc.gpsimd.dma_start(out=din[:], in_=red_in[:])
                nc.gpsimd.collective_compute(
                    "AllReduce", ALU.add,
                    replica_groups=[list(range(NCORES))],
                    ins=[din.opt()], outs=[dout.opt()],
                )
                red = small.tile([CH, 2], F32, tag="red")
                nc.gpsimd.dma_start(out=red[:], in_=dout[:])
                mu = small.tile([CH, 1], F32, tag="mu")
                nc.vector.tensor_scalar_mul(out=mu[:], in0=red[:, 0:1],
                                            scalar1=1.0 / NCORES)
                ex2 = small.tile([CH, 1], F32, tag="ex2")
                nc.vector.tensor_scalar_mul(out=ex2[:], in0=red[:, 1:2],
                                            scalar1=1.0 / NCORES)
                musq = small.tile([CH, 1], F32, tag="musq")
                nc.vector.tensor_mul(out=musq[:], in0=mu[:], in1=mu[:])
                var = small.tile([CH, 1], F32, tag="var")
                nc.vector.tensor_tensor(out=var[:], in0=ex2[:], in1=musq[:],
                                        op=ALU.subtract)
                sd = small.tile([CH, 1], F32, tag="sd")
                nc.scalar.activation(out=sd[:], in_=var[:], func=AF.Sqrt,
                                     bias=eps_t[:], scale=1.0)
                rstd = small.tile([CH, 1], F32, tag="rstd")
                nc.vector.reciprocal(out=rstd[:], in_=sd[:])
                a_t = small.tile([CH, 1], F32, tag="a_t")
                nc.vector.tensor_mul(out=a_t[:], in0=cpack[:, 3 + l:4 + l], in1=rstd[:])
                amu = small.tile([CH, 1], F32, tag="amu")
                nc.vector.tensor_mul(out=amu[:], in0=a_t[:], in1=mu[:])
                c_t = small.tile([CH, 1], F32, tag="c_t")
                nc.vector.tensor_tensor(out=c_t[:], in0=cpack[:, 6 + l:7 + l],
                                        in1=amu[:], op=ALU.subtract)
                # hT = relu(a * mT + c)
                for g in range(G):
                    nc.scalar.activation(out=hT[:, g, :], in_=mT[:, g, :],
                                         func=AF.Relu, bias=c_t[:], scale=a_t[:])

            # ================= outputs + attention =================
            # final transpose pass (h3 natural layout) + h3 output DMA
            for g in range(G):
                pt = transps.tile([CSZ, NK, CH], FP16, tag="pt")
                for k in range(NK):
                    nc.tensor.transpose(
                        pt[:, k, :], hT[:, g, k * CSZ:(k + 1) * CSZ], ident[:])
                nc.vector.tensor_copy(out=hnat[:, g, :, :], in_=pt[:])
                nc.gpsimd.dma_start(
                    out=h3_d[g].rearrange("(k p) c -> p k c", p=CSZ),
                    in_=hnat[:, g, :, :],
                )

            # ghS = sum over nodes per graph (Wq piece 0 is prescaled by 1/NN)
            ghS32 = small.tile([CH, G], F32, tag="ghS32")
            nc.vector.tensor_reduce(out=ghS32[:], in_=hT[:], axis=mybir.AxisListType.X,
                                    op=ALU.add)
            ghS = small.tile([CH, G], FP16, tag="ghS")
            nc.vector.tensor_copy(out=ghS[:], in_=ghS32[:])

            # q_a^T [64k, 16g] = sum_pieces WqP[a,p].T @ rhs_p  (+ bq)
            q_ps = mlpps.tile([CH, NA, G], F32, tag="mlp")
            for a in range(NA):
                for p in range(11):
                    wq_ap = wpack[:, WQP_OFF + 64 * (a * 11 + p):
                                  WQP_OFF + 64 * (a * 11 + p) + 64]
                    rhs = ghS[:] if p == 0 else hT[:, :, p - 1]
                    nc.tensor.matmul(q_ps[:, a, :], wq_ap, rhs,
                                     start=(p == 0), stop=(p == 10))
            qT = pers.tile([CH, NA, G], FP16)
            for a in range(NA):
                nc.scalar.activation(out=qT[:, a, :], in_=q_ps[:, a, :],
                                     func=AF.Identity, bias=cpack[:, 9 + a:10 + a],
                                     scale=1.0)

            # T_a [64c, 16g] = Wk_a @ q_a
            T_ps = mlpps.tile([CH, NA, G], F32, tag="mlp")
            for a in range(NA):
                nc.tensor.matmul(T_ps[:, a, :],
                                 wpack[:, WKT_OFF + 64 * a:WKT_OFF + 64 * a + 64],
                                 qT[:, a, :], start=True, stop=True)
            T_all = pers.tile([CH, NA, G], FP16)
            nc.vector.tensor_copy(out=T_all[:], in_=T_ps[:])

            # per graph: u^T = h^T.T @ T_g ; w~ = exp(u/8) (depot rows zeroed);
            # Z += ones.T @ w~ ; s~^T += hnat.T @ w~
            Z_ps = aggps.tile([1, G, NA], F32, tag="agg")
            s_ps = aggps.tile([CH, G, NA], F32, tag="agg")

            def emit_u(g):
                up = transps.tile([CSZ, NK, NA], F32, tag="pt")
                for k in range(NK):
                    nc.tensor.matmul(up[:, k, :], hT[:, g, k * CSZ:(k + 1) * CSZ],
                                     T_all[:, :, g], start=True, stop=True)
                wt = stream.tile([CSZ, NK, NA], FP16, tag="wt")
                nc.scalar.activation(out=wt[:], in_=up[:], func=AF.Exp, scale=0.125)
                nc.vector.memset(wt[0:NA, 0, :], 0.0)
                return wt

            def emit_sz(g, wt):
                for k in range(NK):
                    nc.tensor.matmul(Z_ps[:, g, :], ones[:], wt[:, k, :],
                                     start=(k == 0), stop=(k == NK - 1))
                for k in range(NK):
                    nc.tensor.matmul(s_ps[:, g, :], hnat[:, g, k, :], wt[:, k, :],
                                     start=(k == 0), stop=(k == NK - 1))

            wt = emit_u(0)
            for g in range(G):
                nwt = emit_u(g + 1) if g + 1 < G else None
                emit_sz(g, wt)
                wt = nwt

            # normalize: sT_sc = s~ / Z  (Z broadcast across partitions via DRAM)
            Zs = small.tile([1, G * NA], F32, tag="Zs")
            nc.vector.tensor_copy(out=Zs[:], in_=Z_ps[0:1, :, :])
            rz = small.tile([1, G * NA], F32, tag="rz")
            nc.vector.reciprocal(out=rz[:], in_=Zs[:])
            rzb = dram.tile([1, G * NA], F32, tag="rzb")
            nc.gpsimd.dma_start(out=rzb[:], in_=rz[:])
            rzB = pers.tile([CH, G * NA], F32)
            rzb_ap = rzb[:]
            bcast = bass.AP(tensor=rzb_ap.tensor, offset=rzb_ap.offset,
                            ap=[[0, CH]] + list(rzb_ap.ap[1:]))
            nc.gpsimd.dma_start(out=rzB[:], in_=bcast)
            sT = pers.tile([CH, G, NA], FP16)
            nc.vector.tensor_tensor(out=sT[:], in0=s_ps[:],
                                    in1=rzB[:].rearrange("c (g a) -> c g a", g=G),
                                    op=ALU.mult)

            # emb_a^T [64v, 16g] = Wv_a.T? -> out = Wv_a.T ... lhsT=Wv_a [c, v]
            emb_ps = mlpps.tile([CH, NA, G], F32, tag="mlp")
            for a in range(NA):
                nc.tensor.matmul(emb_ps[:, a, :],
                                 wpack[:, WV_OFF + 64 * a:WV_OFF + 64 * a + 64],
                                 sT[:, :, a], start=True, stop=True)
            emb_sb = pers.tile([CH, NA, G], F32)
            for a in range(NA):
                nc.scalar.activation(out=emb_sb[:, a, :], in_=emb_ps[:, a, :],
                                     func=AF.Identity, bias=cpack[:, 19 + a:20 + a],
                                     scale=1.0)
            nc.sync.dma_start(out=embT_d[:], in_=emb_sb[:])

    nc.compile()
    return nc


def _prep_host(x, edge_src, edge_dst, gin_params, Wq, bq, Wk, bk, Wv, bv):
    x = np.asarray(x, np.float32)
    src = np.asarray(edge_src, np.int64)
    dst = np.asarray(edge_dst, np.int64)

    g_dst = dst // NN
    assert (src // NN == g_dst).all(), "edges must stay within graphs"
    s_loc = src - (src // NN) * NN
    d_loc = dst - g_dst * NN

    flat = (g_dst * NN + s_loc) * NN + d_loc
    per_core_at = []
    max_cnt = 0
    for c in range(NCORES):
        lo, hi = c * G * NN, (c + 1) * G * NN
        sel = (g_dst >= c * G) & (g_dst < (c + 1) * G)
        sub = flat[sel] - lo * NN
        counts = np.bincount(sub, minlength=G * NN * NN)
        max_cnt = max(max_cnt, int(counts.max()))
        at = counts.reshape(G, NK, CSZ, NN).transpose(0, 2, 1, 3)  # [G,125,8,1000]
        per_core_at.append(np.ascontiguousarray(at.astype(np.float32)))

    # counts up to 16 are exact in fp8-e4m3; fall back to fp16 otherwise
    at_dtype = FP8 if max_cnt <= 16 else FP16
    np_at = mybir.dt.np(at_dtype)
    per_core_at = [a.astype(np_at) for a in per_core_at]

    # per-core x
    per_core_x = []
    for c in range(NCORES):
        xc = x[c * G * NN:(c + 1) * G * NN]
        xnat = np.ascontiguousarray(
            xc.reshape(G, NK, CSZ, CIN).transpose(2, 0, 1, 3)).astype(np.float16)
        xT = np.ascontiguousarray(xc.T).astype(np.float16)
        per_core_x.append((xnat, xT))

    # weights (replicated)
    gp = [[np.asarray(t, np.float32) for t in layer] for layer in gin_params]
    Wq = np.asarray(Wq, np.float32); bq = np.asarray(bq, np.float32)
    Wk = np.asarray(Wk, np.float32); Wv = np.asarray(Wv, np.float32)
    bv = np.asarray(bv, np.float32)

    w10 = gp[0][0].astype(np.float16)                     # [2, 64]
    wcols = []
    wcols.append(gp[1][0]); wcols.append(gp[2][0])        # W1_1, W1_2
    wcols.append(gp[0][2]); wcols.append(gp[1][2]); wcols.append(gp[2][2])  # W2
    for a in range(NA):
        wcols.append(Wk[a].T)                             # WkT [k, c]
    for a in range(NA):
        wcols.append(Wv[a])                               # Wv [c, v]
    for a in range(NA):
        wcols.append(Wq[a][:CH] / float(NN))              # piece 0, prescaled
        for d in range(NA):
            wcols.append(Wq[a][CH + CH * d:CH + CH * (d + 1)])
    wpack = np.concatenate(wcols, axis=1).astype(np.float16)
    assert wpack.shape == (CH, 8640), wpack.shape

    ccols = [gp[0][1], gp[1][1], gp[2][1],                # b1
             gp[0][4], gp[1][4], gp[2][4],                # gamma
             gp[0][5], gp[1][5], gp[2][5]]                # beta
    ccols = [c.reshape(CH, 1) for c in ccols]
    ccols.append(bq.T)                                    # [64, 10]
    ccols.append(bv.T)                                    # [64, 10]
    cpack = np.concatenate(ccols, axis=1).astype(np.float32)
    assert cpack.shape == (CH, 29), cpack.shape

    in_maps = []
    for c in range(NCORES):
        xnat, xT = per_core_x[c]
        in_maps.append({
            "at": per_core_at[c],
            "xnat": xnat,
            "xT": xT,
            "w10": w10,
            "wpack": wpack,
            "cpack": cpack,
        })
    return in_maps, at_dtype


def kernel(x, edge_src, edge_dst, gin_params, Wq, bq, Wk, bk, Wv, bv,
           n_nodes, n_batch):
    global LAST_RESULTS
    assert int(n_nodes) == NN and int(n_batch) == B

    in_maps, at_dtype = _prep_host(x, edge_src, edge_dst, gin_params,
                                   Wq, bq, Wk, bk, Wv, bv)

    key = str(at_dtype)
    if key not in _PROG_CACHE:
        _PROG_CACHE[key] = _build_program(at_dtype)
    nc = _PROG_CACHE[key]

    trace = bool(os.environ.get("BASS_TRACE"))
    res = run_bass_kernel_spmd(nc, in_maps, core_ids=list(range(NCORES)),
                               trace=trace)
    LAST_RESULTS = res

    h3 = np.concatenate([r["h3"] for r in res.results], axis=0)   # [128,1000,64]
    f = np.ascontiguousarray(h3[:, NA:, :])
    emb = np.concatenate(
        [r["embT"].transpose(2, 1, 0) for r in res.results], axis=0)  # [128,10,64]
    return emb, f
